# revision 4
# baseline (speedup 1.0000x reference)
"""VQ codebook lookup (nn_VQ) on 8 TRN2 NeuronCores.

reference: idx = argmin_k ||x_n - e_k||^2 ; out = embeddings[idx]
Equivalent: idx = argmax_k (x_n . e_k - 0.5||e_k||^2)  (||x||^2 is constant per row)

Strategy (data-parallel over N, codebook replicated):
  - Host: shard x into 8 x [62500, 100], pad to [62976, 100] (123 super-tiles
    of 512 rows), split into bf16 hi/lo pairs packed into one [NP, 256] bf16
    array (cols 0:100 x_hi, col 100 = 1.0 bias-aug, 128:228 x_lo). The
    codebook (tiny) is replicated: e.T hi/lo with a bias row (hi/lo split),
    plus natural-layout e hi/lo for the gather matmul.
  - Device, per 512-row super-tile:
      xbar-transpose DMA loads xT (bf16) straight into SBUF
      3x accumulating bf16 matmuls -> scores[n,k] (+bias via aug row), f32 PSUM
      DVE reduce_max + is_ge (broadcast AP) -> exact one-hot bf16 mask
      PE-transpose mask -> maskT ; 2x bf16 matmuls vs e_hi/e_lo -> out rows f32
      DMA out.
  bf16 hi/lo 3-pass matmul gives ~fp32 score precision, so argmax flips vs
  the f32 reference are ~1-in-1e5 near-tie rows.
"""

import sys

sys.path.insert(0, "/opt/trn_rl_repo")
from contextlib import ExitStack

import ml_dtypes
import numpy as np

import concourse.bass as bass
import concourse.bacc as bacc
import concourse.tile as tile
from concourse import mybir
from concourse._compat import with_exitstack
from concourse.bass_utils import run_bass_kernel_spmd

BF = mybir.dt.bfloat16
F32 = mybir.dt.float32
bf16 = ml_dtypes.bfloat16

N_TOTAL = 500_000
D = 100
K = 100
N_CORES = 8
ST = 512  # rows per super-tile (PSUM-bank sized)
N_SHARD = N_TOTAL // N_CORES  # 62500
N_ST = -(-N_SHARD // ST)  # 123 super-tiles
NP = N_ST * ST  # 62976 padded rows per core


@with_exitstack
def _vq_tile_kernel(
    ctx: ExitStack, tc: tile.TileContext, out, xh, eth, etl, eh, el, ident, reps=1
):
    nc = tc.nc

    consts = ctx.enter_context(tc.tile_pool(name="consts", bufs=1))
    eth_s = consts.tile([101, K], BF, tag="eth")
    nc.sync.dma_start(eth_s[:], eth[:])
    etl_s = consts.tile([101, K], BF, tag="etl")
    nc.sync.dma_start(etl_s[:], etl[:])
    eh_s = consts.tile([K, D], BF, tag="eh")
    nc.sync.dma_start(eh_s[:], eh[:])
    el_s = consts.tile([K, D], BF, tag="el")
    nc.sync.dma_start(el_s[:], el[:])
    id_s = consts.tile([128, 128], BF, tag="ident")
    nc.sync.dma_start(id_s[:], ident[:])

    xp = ctx.enter_context(tc.tile_pool(name="xt", bufs=4))
    sp = ctx.enter_context(tc.tile_pool(name="scores", bufs=2, space="PSUM"))
    mp = ctx.enter_context(tc.tile_pool(name="misc", bufs=4))
    mtp = ctx.enter_context(tc.tile_pool(name="maskT", bufs=2, space="PSUM"))
    opp = ctx.enter_context(tc.tile_pool(name="outp", bufs=2, space="PSUM"))
    osb = ctx.enter_context(tc.tile_pool(name="outsb", bufs=4))

    xh_v = xh.rearrange("n (j p) -> n j p", p=128)  # [NP, 2, 128]
    out_v = out.rearrange("(t c p) d -> t p c d", p=128, c=4)

    if reps > 1:
        ctx.enter_context(tc.For_i(0, reps))

    for t in range(N_ST):
        xt = xp.tile([128, 2, ST], BF, tag="xt")
        for j in range(2):
            nc.sync.dma_start(out=xt[:, j], in_=xh_v[bass.ts(t, ST), j], transpose=True)
        scores = sp.tile([128, 4, K], F32, tag="scores")
        for c in range(4):
            hi = xt[0:101, 0, bass.ts(c, 128)]
            lo = xt[0:101, 1, bass.ts(c, 128)]
            nc.tensor.matmul(scores[:, c], hi, eth_s[:], start=True, stop=False)
            nc.tensor.matmul(scores[:, c], hi, etl_s[:], start=False, stop=False)
            nc.tensor.matmul(scores[:, c], lo, eth_s[:], start=False, stop=True)
        maxv = mp.tile([128, 4], F32, tag="maxv")
        nc.vector.tensor_reduce(
            maxv[:], scores[:], axis=mybir.AxisListType.X, op=mybir.AluOpType.max
        )
        mask = mp.tile([128, 4, K], BF, tag="mask")
        mv = maxv[:].rearrange("p (f o) -> p f o", o=1)
        s_ap, m_ap = bass.broadcast_tensor_aps(scores[:], mv)
        nc.vector.tensor_tensor(out=mask[:], in0=s_ap, in1=m_ap, op=mybir.AluOpType.is_ge)
        maskT = mtp.tile([K, 4, 128], BF, tag="maskT")
        for c in range(4):
            nc.tensor.transpose(maskT[:, c], mask[:, c], id_s[:])
        maskTs = mp.tile([K, 4, 128], BF, tag="maskTs")
        nc.scalar.copy(maskTs[:], maskT[:])
        outp = opp.tile([128, 4, D], F32, tag="outp")
        for c in range(4):
            nc.tensor.matmul(outp[:, c], maskTs[:, c], eh_s[:], start=True, stop=False)
            nc.tensor.matmul(outp[:, c], maskTs[:, c], el_s[:], start=False, stop=True)
        outt = osb.tile([128, 4, D], F32, tag="outt")
        nc.scalar.copy(outt[:], outp[:])
        nc.sync.dma_start(out=out_v[t], in_=outt[:])


def build_nc(reps=1):
    nc = bacc.Bacc(
        "TRN2",
        target_bir_lowering=False,
        debug=False,
        enable_asserts=True,
        num_devices=N_CORES,
    )
    out = nc.dram_tensor("out", [NP, D], F32, kind="ExternalOutput").ap()
    xh = nc.dram_tensor("xh", [NP, 256], BF, kind="ExternalInput").ap()
    eth = nc.dram_tensor("eth", [101, K], BF, kind="ExternalInput").ap()
    etl = nc.dram_tensor("etl", [101, K], BF, kind="ExternalInput").ap()
    eh = nc.dram_tensor("eh", [K, D], BF, kind="ExternalInput").ap()
    el = nc.dram_tensor("el", [K, D], BF, kind="ExternalInput").ap()
    ident = nc.dram_tensor("ident", [128, 128], BF, kind="ExternalInput").ap()
    with tile.TileContext(nc) as tc:
        _vq_tile_kernel(tc, out, xh, eth, etl, eh, el, ident, reps=reps)
    nc.compile()
    return nc


def prep_inputs(inputs: np.ndarray, embeddings: np.ndarray):
    """Host-side shard + layout prep. Returns in_maps for the 8 cores."""
    x = np.ascontiguousarray(inputs, dtype=np.float32)
    e = np.ascontiguousarray(embeddings, dtype=np.float32)

    e_hi = e.astype(bf16)
    e_lo = (e - e_hi.astype(np.float32)).astype(bf16)
    bias = (-0.5 * np.sum(e.astype(np.float64) ** 2, axis=1)).astype(np.float32)
    b_hi = bias.astype(bf16)
    b_lo = (bias - b_hi.astype(np.float32)).astype(bf16)
    eth = np.zeros((101, K), dtype=bf16)
    eth[0:D] = e_hi.T
    eth[100] = b_hi
    etl = np.zeros((101, K), dtype=bf16)
    etl[0:D] = e_lo.T
    etl[100] = b_lo
    ident = np.eye(128, dtype=bf16)

    x_hi = x.astype(bf16)
    x_lo = (x - x_hi.astype(np.float32)).astype(bf16)

    in_maps = []
    for i in range(N_CORES):
        lo_r, hi_r = i * N_SHARD, (i + 1) * N_SHARD
        xh = np.zeros((NP, 256), dtype=bf16)
        xh[:N_SHARD, 0:D] = x_hi[lo_r:hi_r]
        xh[:N_SHARD, 100] = 1.0
        xh[:N_SHARD, 128 : 128 + D] = x_lo[lo_r:hi_r]
        in_maps.append(
            {"xh": xh, "eth": eth, "etl": etl, "eh": e_hi, "el": e_lo, "ident": ident}
        )
    return in_maps


_NC_CACHE = None


def kernel(inputs: np.ndarray, embeddings: np.ndarray) -> np.ndarray:
    global _NC_CACHE
    if _NC_CACHE is None:
        _NC_CACHE = build_nc()
    nc = _NC_CACHE
    in_maps = prep_inputs(inputs, embeddings)
    res = run_bass_kernel_spmd(nc, in_maps, core_ids=list(range(N_CORES)))
    shards = [res.results[i]["out"][:N_SHARD] for i in range(N_CORES)]
    return np.ascontiguousarray(np.concatenate(shards, axis=0), dtype=np.float32)

